# revision 18
# baseline (speedup 1.0000x reference)
"""Trainium2 Bass kernel for nn_MessagePassing (gnn_message_passing).

Data-parallel over the molecule dim n: 8 molecules -> 8 NeuronCores, one
molecule per core, params replicated. Everything per-molecule stays on one
core; no collectives.

Math (per molecule, ma=128 atoms, nf=16 RBFs, hid=32):
  d[a,b]   = sqrt(|r_a - r_b|^2 + eps)
  g[f,a,b] = 5 * exp(-(d - mu_f)^2 / sg_f)            (pair mask handled below)
  P[a,b,c] = h[a,:] @ Wc0 + bc + sum_f g[f,a,b] * (h[b,:] @ Wcf)
  S        = silu(P)
  m[a,c]   = mask_a * (sum_b S[a,b,c] - nmasked * silu(A2[a,c]))
  h_new    = h + 0.1 * tanh([m | h] @ Wu + bu)

Key device layout trick: distances are DMA-replicated into a
(quadrant q, half u, rbf f) partition packing so that
  - the RBF expansion runs on all 128 partitions (2 DVE + 1 ACT pass), and
  - each pair-message matmul is a K=32 stationary load of one full
    32-partition quadrant (tile-position aligned), with the sibling half's
    contribution killed by zeros in the moving operand.
Masked (padded) atoms are handled by adding 1e9 to their distance rows
before the RBF (driving g to exactly 0) plus a tiny analytic correction
for the h[a]-only term that survives silu.
"""

import os
import sys

sys.path.insert(0, "/opt/trn_rl_repo")

from contextlib import ExitStack

import numpy as np

import concourse.bass as bass
import concourse.tile as tile
from concourse import bacc, mybir
from concourse.bass_utils import run_bass_kernel_spmd

F32 = mybir.dt.float32
MA, NF, HID = 128, 16, 32
N_CORES = 8
EPS = 1e-12
UPDATE_RATIO = 0.1
DBIG = 1e9  # added to d^2 rows of masked atoms; exp(-(sqrt(DBIG))^2/sg) == 0

AF = mybir.ActivationFunctionType
ALU = mybir.AluOpType


# --------------------------------------------------------------------------
# device program (shared by all cores; per-core data comes via inputs)
# --------------------------------------------------------------------------

_INPUT_SPECS = {
    "lhs_d": [6, MA],      # [-2 r^T (3); |r|^2+eps; ones; 1e9*(1-mask)]
    "rhs_d": [6, MA],      # [r^T (3); ones; |r|^2; ones]
    "h_in": [MA, HID],
    "hT": [HID, MA],
    "Wmg": [HID, 512],     # Wc pair block, columns (g, cl, f) packed
    "Wc0": [HID, HID],
    "bcrep": [MA, HID],    # bc broadcast to all partitions
    "Wu1": [HID, HID],
    "Wu2": [HID, HID],
    "bu_row": [1, HID],
    "maskA": [MA, 1],
    "negnm": [MA, 1],      # -(# masked atoms), replicated
    "bmu": [MA, 1],        # -mu_f in (q,u,f) partition packing
    "bsg": [MA, 1],        # -1/sg_f in (q,u,f) partition packing
}


def _emit(nc, tc, ctx, ins, out_hnew):
    ts = bass.ts

    pool = ctx.enter_context(tc.tile_pool(name="sb", bufs=1))
    ppool = ctx.enter_context(tc.tile_pool(name="ps", bufs=1, space="PSUM"))

    # ---- load inputs to SBUF ----
    sb = {}
    for name, shape in _INPUT_SPECS.items():
        t = pool.tile(shape, F32, tag=name)
        nc.sync.dma_start(t[:], ins[name][:])
        sb[name] = t

    # ---- d^2 via one K=6 matmul (Gram + norms + eps + masked-row bignum) ----
    d2p = ppool.tile([MA, MA], F32, tag="ps_small")
    nc.tensor.matmul(d2p[:], sb["lhs_d"][:], sb["rhs_d"][:], start=True, stop=True)

    # clamp tiny negative cancellation, then sqrt
    dS = pool.tile([MA, MA], F32, tag="dS")
    nc.vector.tensor_scalar_max(dS[:], d2p[:], 0.0)
    dmat = pool.tile([MA, MA], F32, tag="dmat")
    nc.scalar.activation(dmat[:], dS[:], AF.Sqrt)

    # exact self-distance: overwrite diagonal with sqrt(eps)=1e-6
    idx = pool.tile([MA, MA], mybir.dt.int32, tag="idx")
    nc.gpsimd.iota(idx[:], pattern=[[1, MA]], base=0, channel_multiplier=-1)
    eye = pool.tile([MA, MA], mybir.dt.int32, tag="eye")
    nc.vector.tensor_scalar(eye[:], idx[:], 0, None, op0=ALU.is_equal)
    c6 = pool.tile([MA, MA], F32, tag="c6")
    nc.vector.memset(c6[:], 1e-6)
    nc.vector.copy_predicated(dmat[:], eye[:], c6[:])

    # ---- replicate d rows into (q, u, f) packing: dbig[128, 2048] ----
    # dbig[(q,u,f), (bl, a)] = d[32q + 16u + bl, a],  bl in [0,16)
    # bounce d through DRAM so the 16x row replication is a clean
    # 0-step-broadcast read (SBUF-side partition tricks break dep tracking)
    nc.sync.dma_start(ins["scr_d"][:], dmat[:])
    dbig = pool.tile([MA, 16 * MA], F32, tag="dbig")
    for qu in range(8):
        base = 16 * qu
        src = ins["scr_d"][base * MA : (base + 16) * MA]
        src = src.unsqueeze(0).broadcast_to((NF, 16 * MA))
        nc.sync.dma_start(dbig[base : base + 16, :], src)

    # ---- RBF: gbig = 5*exp(-(d - mu_f)^2 / sg_f), chunked for overlap ----
    tg = pool.tile([MA, 16 * MA], F32, tag="tg")
    gbig = pool.tile([MA, 16 * MA], F32, tag="gbig")
    lnb = pool.tile([MA, 1], F32, tag="lnb")
    nc.vector.memset(lnb[:], float(np.log(5.0)))
    n_chunks = 2
    gchunk = 16 * MA // n_chunks
    for j in range(n_chunks):
        cs = slice(j * gchunk, (j + 1) * gchunk)
        nc.vector.tensor_scalar_add(tg[:, cs], dbig[:, cs], sb["bmu"][:])
        nc.vector.tensor_mul(tg[:, cs], tg[:, cs], tg[:, cs])
        nc.scalar.activation(
            gbig[:, cs], tg[:, cs], AF.Exp, bias=lnb[:], scale=sb["bsg"][:]
        )

    # ---- B tensors: Ball[(cl,f), (g, b)] = (h @ Wcf)[b, 8g+cl] ----
    ball = ppool.tile([MA, 512], F32, tag="ball")
    for g in range(4):
        nc.tensor.matmul(
            ball[:, ts(g, MA)],
            sb["Wmg"][:, ts(g, MA)],
            sb["hT"][:],
            start=True,
            stop=True,
        )

    # ---- scatter Ball -> BreshD[(q,u,f), u_sel*512 + c*16 + bl] ----
    # the u_sel != u half of each row stays zero (kills sibling-half terms
    # in the K=32 pair matmuls)
    breshd = pool.tile([MA, 1024], F32, tag="breshd")
    nc.vector.memset(breshd[:], 0.0)
    ball_s = pool.tile([MA, 512], F32, tag="ball_s")
    nc.scalar.copy(ball_s[:], ball[:])
    # hop 1: demote cl from partitions -> X[f, c*128 + b] = Bf[b, c]
    xb = pool.tile([NF, HID * MA], F32, tag="xb")
    xb_r = xb[:].rearrange("f (c b) -> f c b", b=MA)
    for cl in range(8):
        dst = xb_r[:, cl::8, :]  # c in {8g + cl}
        nc.sync.dma_start(dst, ball_s[cl * NF : (cl + 1) * NF, :])
    # hop 2: rows -> (q, u, f) packing, cols -> u*512 + c*16 + bl
    for q in range(4):
        for u in range(2):
            base = 32 * q + 16 * u
            src = xb_r[:, :, base : base + 16]
            nc.sync.dma_start(
                breshd[base : base + NF, 512 * u : 512 * (u + 1)], src
            )

    # ---- A2 = h @ Wc0 + bc ----
    apsum = ppool.tile([MA, HID], F32, tag="ps_small")
    nc.tensor.matmul(apsum[:], sb["hT"][:], sb["Wc0"][:], start=True, stop=True)
    a2 = pool.tile([MA, HID], F32, tag="a2")
    nc.vector.tensor_add(a2[:], apsum[:], sb["bcrep"][:])

    # ---- pair messages, in 4 waves of 32 neighbors (= one quadrant each):
    # P3[a, b*32+c] = sum_f g[f,a,b] * Bf[b,c], then S = silu(P3 + A2) ----
    spre = pool.tile([MA, MA * HID], F32, tag="spre")
    sig = pool.tile([MA, MA * HID], F32, tag="sig")
    smsg = pool.tile([MA, MA * HID], F32, tag="smsg")
    for q in range(4):
        p3w = ppool.tile([MA, 32 * HID], F32, tag="p3", bufs=2)
        for bl_all in range(32):
            b = 32 * q + bl_all
            u, bl = divmod(bl_all, 16)
            lhsT = gbig[32 * q : 32 * (q + 1), ts(bl, MA)]
            # cols u*512 + c*16 + bl, c = 0..31  (stride 16)
            rhs = breshd[32 * q : 32 * (q + 1), :].rearrange(
                "p (us c bl2) -> p us c bl2", us=2, c=HID
            )[:, u, :, bl]
            nc.tensor.matmul(
                p3w[:, ts(bl_all, HID)], lhsT, rhs, start=True, stop=True,
                tile_position=(32 * q, 0),
            )
        cs = slice(q * 32 * HID, (q + 1) * 32 * HID)
        p3_3d = p3w[:].rearrange("p (b c) -> p b c", c=HID)
        a2bj = a2[:].unsqueeze(1).broadcast_to((MA, 32, HID))
        spre_3d = spre[:, cs].rearrange("p (b c) -> p b c", c=HID)
        nc.vector.tensor_tensor(spre_3d, p3_3d, a2bj, op=ALU.add)
        nc.scalar.activation(sig[:, cs], spre[:, cs], AF.Sigmoid)
        nc.vector.tensor_mul(smsg[:, cs], spre[:, cs], sig[:, cs])

    # ---- aggregate over b: doubling tree (c-aligned halves) ----
    red = smsg
    width = MA * HID
    k = 0
    while width > HID:
        half = width // 2
        nxt = pool.tile([MA, half], F32, tag=f"red{k}")
        nc.vector.tensor_add(nxt[:], red[:, 0:half], red[:, half:width])
        red = nxt
        width = half
        k += 1
    m_agg = red  # [MA, HID]

    # ---- masked-b correction + mask_a ----
    siga = pool.tile([MA, HID], F32, tag="siga")
    nc.scalar.activation(siga[:], a2[:], AF.Sigmoid)
    silua = pool.tile([MA, HID], F32, tag="silua")
    nc.vector.tensor_mul(silua[:], a2[:], siga[:])
    m_corr = pool.tile([MA, HID], F32, tag="m_corr")
    nc.vector.scalar_tensor_tensor(
        m_corr[:], silua[:], sb["negnm"][:], m_agg[:], op0=ALU.mult, op1=ALU.add
    )
    nc.vector.tensor_scalar_mul(m_corr[:], m_corr[:], sb["maskA"][:])

    # ---- m^T via 32x32 stream transpose + small DMA ----
    omtb = pool.tile([MA, HID], F32, tag="omtb")
    nc.vector.transpose(omtb[:], m_corr[:])
    m_t = pool.tile([HID, MA], F32, tag="m_t")
    for A in range(4):
        nc.sync.dma_start(
            m_t[:, 32 * A : 32 * (A + 1)], omtb[32 * A : 32 * (A + 1), :]
        )

    # ---- update net: upd = [m | h] @ Wu + bu ----
    ones_row = pool.tile([1, MA], F32, tag="ones_row")
    nc.vector.memset(ones_row[:], 1.0)
    updp = ppool.tile([MA, HID], F32, tag="ps_small")
    nc.tensor.matmul(updp[:], m_t[:], sb["Wu1"][:], start=True, stop=False)
    nc.tensor.matmul(updp[:], sb["hT"][:], sb["Wu2"][:], start=False, stop=False)
    nc.tensor.matmul(
        updp[:], ones_row[:], sb["bu_row"][:], start=False, stop=True
    )
    tanh_t = pool.tile([MA, HID], F32, tag="tanh_t")
    nc.scalar.activation(tanh_t[:], updp[:], AF.Tanh)
    hnew = pool.tile([MA, HID], F32, tag="hnew")
    nc.vector.scalar_tensor_tensor(
        hnew[:], tanh_t[:], UPDATE_RATIO, sb["h_in"][:], op0=ALU.mult, op1=ALU.add
    )
    nc.sync.dma_start(out_hnew[:], hnew[:])


def build_program():
    nc = bacc.Bacc("TRN2", target_bir_lowering=False, debug=False)
    ins = {
        name: nc.dram_tensor(name, shape, F32, kind="ExternalInput").ap()
        for name, shape in _INPUT_SPECS.items()
    }
    ins["scr_d"] = nc.dram_tensor("scr_d", [MA * MA], F32).ap()
    out_hnew = nc.dram_tensor("hnew", [MA, HID], F32, kind="ExternalOutput").ap()
    with tile.TileContext(nc) as tc:
        with ExitStack() as ctx:
            _emit(nc, tc, ctx, ins, out_hnew)
    nc.compile()
    return nc


# --------------------------------------------------------------------------
# host-side prep
# --------------------------------------------------------------------------

def _quf_pack(vec16: np.ndarray) -> np.ndarray:
    """Pack a per-f vector into the (q, u, f) 128-partition layout."""
    out = np.zeros((MA, 1), np.float32)
    for q in range(4):
        for u in range(2):
            base = 32 * q + 16 * u
            out[base : base + NF, 0] = vec16
    return out


def prep_core_inputs(z, r, h, distances, widths, Wc, bc, Wu, bu):
    """Inputs for ONE molecule (z:[128], r:[128,3], h:[128,32])."""
    z = np.asarray(z)
    r = np.asarray(r, np.float32)
    h = np.asarray(h, np.float32)
    mu = np.asarray(distances, np.float32)
    sg = np.asarray(widths, np.float32)
    Wc = np.asarray(Wc, np.float32)
    bc = np.asarray(bc, np.float32)
    Wu = np.asarray(Wu, np.float32)
    bu = np.asarray(bu, np.float32)

    mask = (z > -1).astype(np.float32)  # [128]
    rn2 = np.sum(r * r, axis=1)  # [128]
    ones = np.ones((MA,), np.float32)

    lhs_d = np.stack(
        [-2.0 * r[:, 0], -2.0 * r[:, 1], -2.0 * r[:, 2], rn2 + EPS, ones,
         DBIG * (1.0 - mask)]
    ).astype(np.float32)
    rhs_d = np.stack([r[:, 0], r[:, 1], r[:, 2], ones, rn2, ones]).astype(
        np.float32
    )

    # Wmg[e, g*128 + cl*16 + f] = Wc[32 + 32f + e, 8g + cl]
    wpair = Wc[HID:, :].reshape(NF, HID, HID)  # [f, e, c]
    wmg = np.zeros((HID, 512), np.float32)
    for g in range(4):
        for cl in range(8):
            for f in range(NF):
                wmg[:, g * 128 + cl * 16 + f] = wpair[f, :, 8 * g + cl]

    nmasked = float(MA - mask.sum())
    return {
        "lhs_d": lhs_d,
        "rhs_d": rhs_d,
        "h_in": h,
        "hT": np.ascontiguousarray(h.T),
        "Wmg": wmg,
        "Wc0": np.ascontiguousarray(Wc[:HID, :]),
        "bcrep": np.broadcast_to(bc, (MA, HID)).copy(),
        "Wu1": np.ascontiguousarray(Wu[:HID, :]),
        "Wu2": np.ascontiguousarray(Wu[HID:, :]),
        "bu_row": bu.reshape(1, HID).copy(),
        "maskA": mask.reshape(MA, 1).copy(),
        "negnm": np.full((MA, 1), -nmasked, np.float32),
        "bmu": _quf_pack(-mu),
        "bsg": _quf_pack(-1.0 / sg),
    }


_NC_CACHE = None


def kernel(z, r, h, distances, widths, Wc, bc, Wu, bu):
    global _NC_CACHE
    z = np.asarray(z)
    r = np.asarray(r)
    h = np.asarray(h)
    n = z.shape[0]
    assert n == N_CORES

    in_maps = [
        prep_core_inputs(z[i], r[i], h[i], distances, widths, Wc, bc, Wu, bu)
        for i in range(n)
    ]

    if _NC_CACHE is None:
        _NC_CACHE = build_program()
    nc = _NC_CACHE

    res = run_bass_kernel_spmd(nc, in_maps, list(range(N_CORES)))
    h_new = np.stack([res.results[i]["hnew"] for i in range(n)]).astype(np.float32)
    return (z, np.asarray(r, np.float32), h_new)


# --------------------------------------------------------------------------
# CoreSim path for fast correctness debugging (not used by the grader)
# --------------------------------------------------------------------------

def run_sim_core(in_map):
    from concourse.bass_interp import CoreSim

    nc = build_program()
    sim = CoreSim(nc)
    for name, val in in_map.items():
        sim.tensor(name)[:] = val
    sim.simulate(check_with_hw=False)
    return np.array(sim.tensor("hnew"))


# revision 22
# speedup vs baseline: 1.0698x; 1.0698x over previous
"""Trainium2 Bass kernel for nn_MessagePassing (gnn_message_passing).

Data-parallel over the molecule dim n: 8 molecules -> 8 NeuronCores, one
molecule per core, params replicated. No collectives.

Math (per molecule, ma=128 atoms, nf=16 RBFs, hid=32):
  d[a,b]   = sqrt(|r_a - r_b|^2 + eps)
  g[f,a,b] = 5 * exp(-(d - mu_f)^2 / sg_f)
  P[a,b,c] = h[a,:] @ Wc0 + bc + sum_f g[f,a,b] * (h[b,:] @ Wcf)
  S        = silu(P)
  m[a,c]   = mask_a * (sum_b S[a,b,c] - nmasked * silu(A2[a,c]))
  h_new    = h + 0.1 * tanh([m | h] @ Wu + bu)

Device layout: distances are replicated (via a DRAM bounce) into a
(quadrant q, half u, rbf f) partition packing; the RBF expansion runs on
all 128 partitions; each pair-message matmul is K=32 over one aligned
32-partition quadrant with the sibling half's moving operand zeroed.
The 128 pair matmuls are issued in two 4-bank PSUM mega-waves with the
issue order rotating across the four row-tile bands so independent PE
tiles overlap. Masked (padded) atoms get +1e9 on their distance rows
(g -> exactly 0) plus an analytic correction for the h[a]-only term.
"""

import os
import sys

sys.path.insert(0, "/opt/trn_rl_repo")

from contextlib import ExitStack

import numpy as np

import concourse.bass as bass
import concourse.tile as tile
from concourse import bacc, mybir
from concourse.bass_utils import run_bass_kernel_spmd

F32 = mybir.dt.float32
MA, NF, HID = 128, 16, 32
N_CORES = 8
EPS = 1e-12
UPDATE_RATIO = 0.1
DBIG = 1e9

AF = mybir.ActivationFunctionType
ALU = mybir.AluOpType

# input blobs: A = 128-row tensors, B = 32-row tensors, C = 6-row tensors
BLOBA_COLS = 68    # h(32) bcrep(32) maskA(1) negnm(1) bmu(1) bsg(1)
BLOBB_COLS = 736   # hT(128) Wmg(512) Wc0(32) Wu1(32) Wu2(32)
BLOBC_COLS = 288   # lhs_d(128) rhs_d(128) bu_row(32, row 0 only)


def _emit(nc, tc, ctx, ins, out_hnew):
    ts = bass.ts
    pool = ctx.enter_context(tc.tile_pool(name="sb", bufs=1))

    # ---- load the three input blobs ----
    blobA = pool.tile([MA, BLOBA_COLS], F32, tag="blobA")
    blobB = pool.tile([HID, BLOBB_COLS], F32, tag="blobB")
    blobC = pool.tile([6, BLOBC_COLS], F32, tag="blobC")
    nc.sync.dma_start(blobA[:], ins["blobA"][:])
    nc.sync.dma_start(blobB[:], ins["blobB"][:])
    nc.sync.dma_start(blobC[:], ins["blobC"][:])

    h_in = blobA[:, 0:32]
    bcrep = blobA[:, 32:64]
    maskA = blobA[:, 64:65]
    negnm = blobA[:, 65:66]
    bmu = blobA[:, 66:67]
    bsg = blobA[:, 67:68]
    hT = blobB[:, 0:128]
    wmg = blobB[:, 128:640]
    wc0 = blobB[:, 640:672]
    wu1 = blobB[:, 672:704]
    wu2 = blobB[:, 704:736]
    lhs_d = blobC[:, 0:128]
    rhs_d = blobC[:, 128:256]
    bu_row = blobC[0:1, 256:288]

    with tc.tile_pool(name="ps1", bufs=1, space="PSUM") as pp1:
        # ---- d^2 via one K=6 matmul ----
        d2p = pp1.tile([MA, MA], F32, tag="d2p")
        nc.tensor.matmul(d2p[:], lhs_d, rhs_d, start=True, stop=True)

        dS = pool.tile([MA, MA], F32, tag="dS")
        nc.vector.tensor_scalar_max(dS[:], d2p[:], 0.0)
        dmat = pool.tile([MA, MA], F32, tag="dmat")
        nc.scalar.activation(dmat[:], dS[:], AF.Sqrt)

        # exact self-distance: diagonal <- sqrt(eps) = 1e-6
        idx = pool.tile([MA, MA], mybir.dt.int32, tag="idx")
        nc.gpsimd.iota(idx[:], pattern=[[1, MA]], base=0, channel_multiplier=-1)
        eye = pool.tile([MA, MA], mybir.dt.int32, tag="eye")
        nc.vector.tensor_scalar(eye[:], idx[:], 0, None, op0=ALU.is_equal)
        c6 = pool.tile([MA, MA], F32, tag="c6")
        nc.vector.memset(c6[:], 1e-6)
        nc.vector.copy_predicated(dmat[:], eye[:], c6[:])

        # ---- replicate d rows into (q,u,f) packing via DRAM bounce ----
        nc.sync.dma_start(ins["scr_d"][:], dmat[:])
        dbig = pool.tile([MA, 16 * MA], F32, tag="dbig")
        for qu in range(8):
            base = 16 * qu
            src = ins["scr_d"][base * MA : (base + 16) * MA]
            src = src.unsqueeze(0).broadcast_to((NF, 16 * MA))
            nc.sync.dma_start(dbig[base : base + 16, :], src)

        # ---- RBF: gbig = 5*exp(-(d - mu_f)^2 / sg_f) in 2 column chunks ----
        tg = pool.tile([MA, 16 * MA], F32, tag="tg")
        gbig = pool.tile([MA, 16 * MA], F32, tag="gbig")
        lnb = pool.tile([MA, 1], F32, tag="lnb")
        nc.vector.memset(lnb[:], float(np.log(5.0)))
        for j in range(2):
            cs = slice(j * 1024, (j + 1) * 1024)
            nc.vector.tensor_scalar_add(tg[:, cs], dbig[:, cs], bmu)
            nc.vector.tensor_mul(tg[:, cs], tg[:, cs], tg[:, cs])
            nc.scalar.activation(gbig[:, cs], tg[:, cs], AF.Exp, bias=lnb[:],
                                 scale=bsg)

        # ---- B tensors: Ball[(cl,f), (g, b)] = (h @ Wcf)[b, 8g+cl] ----
        ball = pp1.tile([MA, 512], F32, tag="ball")
        for g in range(4):
            nc.tensor.matmul(ball[:, ts(g, MA)], wmg[:, ts(g, MA)], hT,
                             start=True, stop=True)
        ball_s = pool.tile([MA, 512], F32, tag="ball_s")
        nc.vector.tensor_copy(ball_s[:], ball[:])

        # ---- scatter Ball -> BreshD[(q,u,f), u*512 + c*16 + bl] (SWDGE;
        # the u_sel != u half of each row stays zero to kill sibling terms
        # in the K=32 pair matmuls) ----
        breshd = pool.tile([MA, 1024], F32, tag="breshd")
        nc.vector.memset(breshd[:], 0.0)
        for cl in range(8):
            src = ball_s[cl * NF : (cl + 1) * NF, :].rearrange(
                "f (g b) -> f g b", g=4
            )
            for qu in range(8):
                q, u = divmod(qu, 2)
                rows = slice(32 * q + 16 * u, 32 * q + 16 * u + NF)
                dst = breshd[rows, 512 * u : 512 * (u + 1)].rearrange(
                    "f (g cl bl) -> f g cl bl", g=4, cl=8
                )[:, :, cl, :]
                nc.gpsimd.dma_start(
                    dst, src[:, :, 32 * q + 16 * u : 32 * q + 16 * u + 16]
                )

        # ---- A2 = h @ Wc0 + bc ----
        apsum = pp1.tile([MA, HID], F32, tag="apsum")
        nc.tensor.matmul(apsum[:], hT, wc0, start=True, stop=True)
        a2 = pool.tile([MA, HID], F32, tag="a2")
        nc.vector.tensor_add(a2[:], apsum[:], bcrep)

    # ---- pair messages: 2 mega-waves x 64 neighbors, band-rotated issue.
    # wave w, column block col(qu, bl_l, c) = qu*256 + bl_l*32 + c where
    # b = 32q + 16u + (8w + bl_l): band q owns PSUM bank q of the wave. ----
    m_part = []
    with tc.tile_pool(name="ps2", bufs=1, space="PSUM") as pp2:
        for w in range(2):
            p3w = pp2.tile([MA, 2048], F32, tag="p3", bufs=2)
            for bl_l in range(8):
                bl = 8 * w + bl_l  # bl within each 16-atom half, in [0,16)
                for u in range(2):
                    for q in range(4):  # band rotates fastest
                        lhsT = gbig[32 * q : 32 * (q + 1), ts(bl, MA)]
                        rhs = breshd[32 * q : 32 * (q + 1), :].rearrange(
                            "p (us c bl2) -> p us c bl2", us=2, c=HID
                        )[:, u, :, bl]
                        qu = 2 * q + u
                        nc.tensor.matmul(
                            p3w[:, 256 * qu + 32 * bl_l : 256 * qu + 32 * bl_l + 32],
                            lhsT, rhs, start=True, stop=True,
                            tile_position=(32 * q, 0),
                        )
            # wave post-processing: S = silu(P3 + A2), then partial reduce
            spre = pool.tile([MA, 2048], F32, tag="spre", bufs=2)
            sig = pool.tile([MA, 2048], F32, tag="sig", bufs=2)
            smsg = pool.tile([MA, 2048], F32, tag="smsg", bufs=2)
            p3_3d = p3w[:].rearrange("p (b c) -> p b c", c=HID)
            a2b = a2.unsqueeze(1).broadcast_to((MA, 64, HID))
            spre_3d = spre[:].rearrange("p (b c) -> p b c", c=HID)
            nc.vector.tensor_tensor(spre_3d, p3_3d, a2b, op=ALU.add)
            nc.scalar.activation(sig[:], spre[:], AF.Sigmoid)
            nc.vector.tensor_mul(smsg[:], spre[:], sig[:])
            # halving tree on gpsimd (keeps DVE free for the next wave)
            red = smsg
            width = 2048
            k = 0
            while width > HID:
                half = width // 2
                nxt = pool.tile([MA, half], F32, tag=f"red{w}_{k}")
                nc.gpsimd.tensor_add(nxt[:], red[:, 0:half], red[:, half:width])
                red = nxt
                width = half
                k += 1
            m_part.append(red)

    m_agg = pool.tile([MA, HID], F32, tag="m_agg")
    nc.vector.tensor_add(m_agg[:], m_part[0][:], m_part[1][:])

    # ---- masked-b correction + mask_a ----
    siga = pool.tile([MA, HID], F32, tag="siga")
    nc.scalar.activation(siga[:], a2[:], AF.Sigmoid)
    silua = pool.tile([MA, HID], F32, tag="silua")
    nc.vector.tensor_mul(silua[:], a2[:], siga[:])
    m_corr = pool.tile([MA, HID], F32, tag="m_corr")
    nc.vector.scalar_tensor_tensor(
        m_corr[:], silua[:], negnm, m_agg[:], op0=ALU.mult, op1=ALU.add
    )
    nc.vector.tensor_scalar_mul(m_corr[:], m_corr[:], maskA)

    # ---- m^T via 32x32 stream transpose + 4 block DMAs ----
    omtb = pool.tile([MA, HID], F32, tag="omtb")
    nc.vector.transpose(omtb[:], m_corr[:])
    m_t = pool.tile([HID, MA], F32, tag="m_t")
    for A in range(4):
        nc.sync.dma_start(
            m_t[:, 32 * A : 32 * (A + 1)], omtb[32 * A : 32 * (A + 1), :]
        )

    # ---- update net ----
    ones_row = pool.tile([1, MA], F32, tag="ones_row")
    nc.vector.memset(ones_row[:], 1.0)
    with tc.tile_pool(name="ps3", bufs=1, space="PSUM") as pp3:
        updp = pp3.tile([MA, HID], F32, tag="updp")
        nc.tensor.matmul(updp[:], m_t[:], wu1, start=True, stop=False)
        nc.tensor.matmul(updp[:], hT, wu2, start=False, stop=False)
        nc.tensor.matmul(updp[:], ones_row[:], bu_row, start=False, stop=True)
        tanh_t = pool.tile([MA, HID], F32, tag="tanh_t")
        nc.scalar.activation(tanh_t[:], updp[:], AF.Tanh)
    hnew = pool.tile([MA, HID], F32, tag="hnew")
    nc.vector.scalar_tensor_tensor(
        hnew[:], tanh_t[:], UPDATE_RATIO, h_in, op0=ALU.mult, op1=ALU.add
    )
    nc.sync.dma_start(out_hnew[:], hnew[:])


def build_program():
    nc = bacc.Bacc("TRN2", target_bir_lowering=False, debug=False)
    ins = {
        "blobA": nc.dram_tensor("blobA", [MA, BLOBA_COLS], F32,
                                kind="ExternalInput").ap(),
        "blobB": nc.dram_tensor("blobB", [HID, BLOBB_COLS], F32,
                                kind="ExternalInput").ap(),
        "blobC": nc.dram_tensor("blobC", [6, BLOBC_COLS], F32,
                                kind="ExternalInput").ap(),
        "scr_d": nc.dram_tensor("scr_d", [MA * MA], F32).ap(),
    }
    out_hnew = nc.dram_tensor("hnew", [MA, HID], F32, kind="ExternalOutput").ap()
    with tile.TileContext(nc) as tc:
        with ExitStack() as ctx:
            _emit(nc, tc, ctx, ins, out_hnew)
    nc.compile()
    return nc


# --------------------------------------------------------------------------
# host-side prep
# --------------------------------------------------------------------------

def _quf_pack(vec16: np.ndarray) -> np.ndarray:
    out = np.zeros((MA,), np.float32)
    for q in range(4):
        for u in range(2):
            base = 32 * q + 16 * u
            out[base : base + NF] = vec16
    return out


def prep_core_inputs(z, r, h, distances, widths, Wc, bc, Wu, bu):
    z = np.asarray(z)
    r = np.asarray(r, np.float32)
    h = np.asarray(h, np.float32)
    mu = np.asarray(distances, np.float32)
    sg = np.asarray(widths, np.float32)
    Wc = np.asarray(Wc, np.float32)
    bc = np.asarray(bc, np.float32)
    Wu = np.asarray(Wu, np.float32)
    bu = np.asarray(bu, np.float32)

    mask = (z > -1).astype(np.float32)
    rn2 = np.sum(r * r, axis=1)
    ones = np.ones((MA,), np.float32)

    blobA = np.zeros((MA, BLOBA_COLS), np.float32)
    blobA[:, 0:32] = h
    blobA[:, 32:64] = np.broadcast_to(bc, (MA, HID))
    blobA[:, 64] = mask
    blobA[:, 65] = -(MA - mask.sum())
    blobA[:, 66] = _quf_pack(-mu)
    blobA[:, 67] = _quf_pack(-1.0 / sg)

    wpair = Wc[HID:, :].reshape(NF, HID, HID)  # [f, e, c]
    wmg = np.zeros((HID, 512), np.float32)
    for g in range(4):
        for cl in range(8):
            for f in range(NF):
                wmg[:, g * 128 + cl * 16 + f] = wpair[f, :, 8 * g + cl]

    blobB = np.zeros((HID, BLOBB_COLS), np.float32)
    blobB[:, 0:128] = h.T
    blobB[:, 128:640] = wmg
    blobB[:, 640:672] = Wc[:HID, :]
    blobB[:, 672:704] = Wu[:HID, :]
    blobB[:, 704:736] = Wu[HID:, :]

    blobC = np.zeros((6, BLOBC_COLS), np.float32)
    blobC[0:3, 0:128] = -2.0 * r.T
    blobC[3, 0:128] = rn2 + EPS
    blobC[4, 0:128] = ones
    blobC[5, 0:128] = DBIG * (1.0 - mask)
    blobC[0:3, 128:256] = r.T
    blobC[3, 128:256] = ones
    blobC[4, 128:256] = rn2
    blobC[5, 128:256] = ones
    blobC[0, 256:288] = bu

    return {"blobA": blobA, "blobB": blobB, "blobC": blobC}


_NC_CACHE = None


def kernel(z, r, h, distances, widths, Wc, bc, Wu, bu):
    global _NC_CACHE
    z = np.asarray(z)
    r = np.asarray(r)
    h = np.asarray(h)
    n = z.shape[0]
    assert n == N_CORES

    in_maps = [
        prep_core_inputs(z[i], r[i], h[i], distances, widths, Wc, bc, Wu, bu)
        for i in range(n)
    ]
    if _NC_CACHE is None:
        _NC_CACHE = build_program()
    res = run_bass_kernel_spmd(_NC_CACHE, in_maps, list(range(N_CORES)))
    h_new = np.stack([res.results[i]["hnew"] for i in range(n)]).astype(np.float32)
    return (z, np.asarray(r, np.float32), h_new)


def run_sim_core(in_map):
    from concourse.bass_interp import CoreSim

    nc = build_program()
    sim = CoreSim(nc)
    for name, val in in_map.items():
        sim.tensor(name)[:] = val
    sim.simulate(check_with_hw=False)
    return np.array(sim.tensor("hnew"))
